# revision 1
# baseline (speedup 1.0000x reference)
"""Trainium2 Bass kernel for nn_DefAddkeysTransformer.

Sharding: one attention head per NeuronCore (8 heads / 8 cores).  Each core
gathers its head's deformable keys, computes the (reshape-scrambled) level
attention scores, the add_keys attention, a max-free softmax, and its head's
output contribution in transposed layout (C, Lq).  Host sums the 8 partial
outputs (the reference's per-head accumulation) and transposes back.
"""
import sys

sys.path.insert(0, '/opt/trn_rl_repo')

from contextlib import ExitStack

import numpy as np

import concourse.bass as bass
import concourse.tile as tile
from concourse import bacc
from concourse import mybir
from concourse.bass_utils import run_bass_kernel_spmd
from concourse.masks import make_identity

C = 256
H = 8
L = 4
P = 4
LQ = 1024
LX = 256
LEN_IN = 13294
NT = LQ // 128          # 8 query tiles
F32 = mybir.dt.float32
F32R = mybir.dt.float32r
F16 = mybir.dt.float16
I32 = mybir.dt.int32


def build_kernel(nc: bass.Bass, tc: tile.TileContext, ctx: ExitStack, debug=False):
    # ---------------- DRAM I/O ----------------
    d_qT = nc.dram_tensor("qT", [C, LQ], F32, kind="ExternalInput").ap()
    d_qT16 = nc.dram_tensor("qT16", [C, LQ], F16, kind="ExternalInput").ap()
    d_flat = nc.dram_tensor("flatten16", [LEN_IN, C], F16, kind="ExternalInput").ap()
    d_rpq = nc.dram_tensor("rp_q", [128, 256], F32, kind="ExternalInput").ap()
    d_invwh = nc.dram_tensor("invwh", [128, 256], F32, kind="ExternalInput").ap()
    d_issf = nc.dram_tensor("issf", [128, 256], F32, kind="ExternalInput").ap()
    d_wh32 = nc.dram_tensor("wh32", [128, 256], F32, kind="ExternalInput").ap()
    d_boff = nc.dram_tensor("boff", [128, 256], F32, kind="ExternalInput").ap()
    d_hl = nc.dram_tensor("hl", [128, 128], I32, kind="ExternalInput").ap()
    d_lvst = nc.dram_tensor("lvst", [128, 128], I32, kind="ExternalInput").ap()
    d_woffT = nc.dram_tensor("woffT", [C, 32], F32, kind="ExternalInput").ap()
    d_wattnT = nc.dram_tensor("wattnT", [5, C, C], F32R, kind="ExternalInput").ap()
    d_addkT = nc.dram_tensor("addkT", [C, LX], F32R, kind="ExternalInput").ap()
    d_wvT16 = nc.dram_tensor("wvT16", [C, C], F16, kind="ExternalInput").ap()
    d_wv2T = nc.dram_tensor("wv2T", [C, C], F32R, kind="ExternalInput").ap()
    d_bvd = nc.dram_tensor("bvd", [1, C], F32R, kind="ExternalInput").ap()
    d_bv2 = nc.dram_tensor("bv2", [C, 1], F32, kind="ExternalInput").ap()
    d_wmix = nc.dram_tensor("wmix_r", [C, 9], F32, kind="ExternalInput").ap()
    d_flag = nc.dram_tensor("flag", [128, 1], F32, kind="ExternalInput").ap()
    d_dmask = nc.dram_tensor("dmask", [128, 512], F32R, kind="ExternalInput").ap()
    d_oh48 = nc.dram_tensor("oh48", [128, 8], F32R, kind="ExternalInput").ap()
    d_out = nc.dram_tensor("outT", [C, LQ], F32, kind="ExternalOutput").ap()
    if debug:
        d_dbg_flat = nc.dram_tensor("dbg_flat", [128, 8, 16], I32, kind="ExternalOutput").ap()
        d_dbg_sall = nc.dram_tensor("dbg_sall", [4, LQ], F32, kind="ExternalOutput").ap()
        d_dbg_wq = nc.dram_tensor("dbg_wq", [LQ, 4], F32, kind="ExternalOutput").ap()
        d_dbg_z = nc.dram_tensor("dbg_z", [LQ, 1], F32, kind="ExternalOutput").ap()
        d_dbg_v = nc.dram_tensor("dbg_v", [LQ, C], F32, kind="ExternalOutput").ap()
        d_dbg_wadd = nc.dram_tensor("dbg_wadd", [128, LX], F32, kind="ExternalOutput").ap()

    # ---------------- pools ----------------
    cst = ctx.enter_context(tc.tile_pool(name="cst", bufs=1))
    gpool = ctx.enter_context(tc.tile_pool(name="gpool", bufs=32))
    wrk = ctx.enter_context(tc.tile_pool(name="wrk", bufs=3))
    stsb = ctx.enter_context(tc.tile_pool(name="stsb", bufs=2))
    ps_st = ctx.enter_context(tc.tile_pool(name="ps_st", bufs=2, space="PSUM"))
    ps_c = ctx.enter_context(tc.tile_pool(name="ps_c", bufs=2, space="PSUM"))
    ps_v = ctx.enter_context(tc.tile_pool(name="ps_v", bufs=1, space="PSUM"))
    ps_tp = ctx.enter_context(tc.tile_pool(name="ps_tp", bufs=1, space="PSUM"))
    ps_o = ctx.enter_context(tc.tile_pool(name="ps_o", bufs=1, space="PSUM"))

    def csttile(shape, dtype=F32, tag=None):
        return cst.tile(shape, dtype, tag=tag, name=tag)

    # ------- phase 0: only what the gather indices need, loaded first -------
    WoffT = [csttile([128, 32], tag=f"wofft{ch}") for ch in range(2)]
    for ch in range(2):
        nc.sync.dma_start(WoffT[ch][:], d_woffT[128 * ch:128 * ch + 128, :])
    INVWH = csttile([128, 256], tag="invwh")
    nc.sync.dma_start(INVWH[:], d_invwh[:])
    ISSF = csttile([128, 256], tag="issf")
    nc.sync.dma_start(ISSF[:], d_issf[:])
    WH32 = csttile([128, 256], tag="wh32")
    nc.sync.dma_start(WH32[:], d_wh32[:])
    BOFF = csttile([128, 256], tag="boff")
    nc.sync.dma_start(BOFF[:], d_boff[:])
    HL = csttile([128, 128], I32, tag="hl")
    nc.sync.dma_start(HL[:], d_hl[:])
    LVST = csttile([128, 128], I32, tag="lvst")
    nc.sync.dma_start(LVST[:], d_lvst[:])
    RPB = csttile([128, 256], tag="rpb")
    QTF = [csttile([128, LQ], tag=f"qtf{ch}") for ch in range(2)]
    OFFALL = csttile([128, 256], tag="offall")
    FLATB = csttile([128, 128], I32, tag="flatb")

    def off_tile(t):
        qsl = slice(128 * t, 128 * t + 128)
        for ch in range(2):
            nc.sync.dma_start(QTF[ch][:, qsl], d_qT[128 * ch:128 * ch + 128, qsl])
        pof = ps_c.tile([128, 32], F32, tag="pc", name="pc")
        for ch in range(2):
            nc.tensor.matmul(pof[:], lhsT=QTF[ch][:, qsl], rhs=WoffT[ch][:],
                             start=(ch == 0), stop=(ch == 1))
        nc.vector.tensor_copy(OFFALL[:, 32 * t:32 * t + 32], pof[:])

    def idx_chain(hf):
        # faithful fp order: (q @ W.T + b), then x/wh (Newton-corrected
        # reciprocal multiply ~ IEEE division), then + rp; exact truncation.
        csl = slice(128 * hf, 128 * hf + 128)
        ksl = slice(64 * hf, 64 * hf + 64)
        t0 = wrk.tile([128, 128], F32, tag="ix0", name="ix0")
        nc.vector.tensor_tensor(out=t0[:], in0=OFFALL[:, csl], in1=BOFF[:, csl],
                                op=mybir.AluOpType.add)
        t1 = wrk.tile([128, 128], F32, tag="ix1", name="ix1")
        nc.vector.tensor_tensor(out=t1[:], in0=t0[:], in1=INVWH[:, csl],
                                op=mybir.AluOpType.mult)
        te = wrk.tile([128, 128], F32, tag="ixe", name="ixe")
        nc.vector.tensor_tensor(out=te[:], in0=t1[:], in1=WH32[:, csl],
                                op=mybir.AluOpType.mult)
        nc.vector.tensor_tensor(out=te[:], in0=t0[:], in1=te[:],
                                op=mybir.AluOpType.subtract)
        nc.vector.tensor_tensor(out=te[:], in0=te[:], in1=INVWH[:, csl],
                                op=mybir.AluOpType.mult)
        nc.vector.tensor_tensor(out=t1[:], in0=t1[:], in1=te[:],
                                op=mybir.AluOpType.add)
        nc.vector.tensor_tensor(out=t1[:], in0=t1[:], in1=RPB[:, csl],
                                op=mybir.AluOpType.add)
        nc.vector.tensor_scalar(out=t1[:], in0=t1[:], scalar1=0.999, scalar2=0.0,
                                op0=mybir.AluOpType.min, op1=mybir.AluOpType.max)
        nc.vector.tensor_tensor(out=t1[:], in0=t1[:], in1=ISSF[:, csl],
                                op=mybir.AluOpType.mult)
        ti = wrk.tile([128, 128], I32, tag="ix2", name="ix2")
        nc.vector.tensor_copy(ti[:], t1[:])      # f32 -> i32 (rounds on HW)
        fb = wrk.tile([128, 128], F32, tag="ixf", name="ixf")
        nc.vector.tensor_copy(fb[:], ti[:])
        gtf = wrk.tile([128, 128], F32, tag="ixg", name="ixg")
        nc.vector.tensor_tensor(out=gtf[:], in0=fb[:], in1=t1[:],
                                op=mybir.AluOpType.is_gt)
        gti = wrk.tile([128, 128], I32, tag="ixh", name="ixh")
        nc.vector.tensor_copy(gti[:], gtf[:])
        nc.vector.tensor_tensor(out=ti[:], in0=ti[:], in1=gti[:],
                                op=mybir.AluOpType.subtract)
        iv = ti[:].rearrange("p (k two) -> p k two", two=2)
        nc.vector.tensor_tensor(out=FLATB[:, ksl], in0=iv[:, :, 1],
                                in1=HL[:, ksl], op=mybir.AluOpType.mult)
        nc.vector.tensor_tensor(out=FLATB[:, ksl], in0=FLATB[:, ksl],
                                in1=iv[:, :, 0], op=mybir.AluOpType.add)
        nc.vector.tensor_tensor(out=FLATB[:, ksl], in0=FLATB[:, ksl],
                                in1=LVST[:, ksl], op=mybir.AluOpType.add)

    nc.sync.dma_start(RPB[:], d_rpq[:])
    for t in range(4):
        off_tile(t)
    idx_chain(0)
    for t in range(4, NT):
        off_tile(t)
    idx_chain(1)
    if debug:
        nc.sync.dma_start(
            d_dbg_flat[:],
            FLATB[:].rearrange("p (t k) -> p t k", t=NT))

    # ------- phase 1: ALL gathers (the GPSIMD descriptor-rate spine) --------
    G4 = [[gpool.tile([128, 4 * C], F16, tag="g", name="g")
           for t in range(NT)] for lvl in range(L)]
    for lvl in range(L):
        for t in range(NT):
            for p in range(P):
                col = 16 * t + 4 * lvl + p
                nc.gpsimd.indirect_dma_start(
                    out=G4[lvl][t][:, 256 * p:256 * p + 256], out_offset=None,
                    in_=d_flat[:],
                    in_offset=bass.IndirectOffsetOnAxis(
                        ap=FLATB[:, col:col + 1], axis=0),
                )

    # ------- phase 2: remaining weight loads + addk branch ------------------
    QT16 = [csttile([128, LQ], F16, tag=f"qt16_{ch}") for ch in range(2)]
    for ch in range(2):
        nc.sync.dma_start(QT16[ch][:], d_qT16[128 * ch:128 * ch + 128, :])
    IDENT = csttile([128, 128], tag="ident")
    make_identity(nc, IDENT[:])
    IDENT16 = csttile([128, 128], F16, tag="ident16")
    nc.vector.tensor_copy(IDENT16[:], IDENT[:])
    DMASK = csttile([128, 512], F32R, tag="dmask")
    nc.sync.dma_start(DMASK[:], d_dmask[:])
    OH48 = csttile([128, 8], F32R, tag="oh48")
    nc.sync.dma_start(OH48[:], d_oh48[:])
    FLG = csttile([128, 1], tag="flg")
    nc.sync.dma_start(FLG[:], d_flag[:])
    BVD = csttile([1, C], F32R, tag="bvd")
    nc.sync.dma_start(BVD[:], d_bvd[:])
    NEG16 = csttile([128, 1], tag="neg16")
    nc.vector.memset(NEG16[:], -16.0)
    SimT = [[csttile([128, C], F32R, tag=f"sim{i}_{ch}") for ch in range(2)]
            for i in range(5)]
    for i in range(5):
        for ch in range(2):
            nc.sync.dma_start(SimT[i][ch][:], d_wattnT[i, 128 * ch:128 * ch + 128, :])
    AddkT = [csttile([128, LX], F32R, tag=f"addkt{ch}") for ch in range(2)]
    WvT16 = [csttile([128, C], F16, tag=f"wvt{ch}") for ch in range(2)]
    Wv2T = [csttile([128, C], F32R, tag=f"wv2t{ch}") for ch in range(2)]
    WM = [csttile([128, 9], tag=f"wm{ch}") for ch in range(2)]
    BV2 = [csttile([128, 1], tag=f"bv2{ch}") for ch in range(2)]
    for ch in range(2):
        sl = slice(128 * ch, 128 * ch + 128)
        nc.sync.dma_start(AddkT[ch][:], d_addkT[sl, :])
        nc.sync.dma_start(WvT16[ch][:], d_wvT16[sl, :])
        nc.sync.dma_start(Wv2T[ch][:], d_wv2T[sl, :])
        nc.sync.dma_start(WM[ch][:], d_wmix[sl, :])
        nc.sync.dma_start(BV2[ch][:], d_bv2[sl, :])

    # head_w softmax over the 9 mixture logits
    HWH = []
    BASE = []
    BV2HW = []
    for ch in range(2):
        mx = wrk.tile([128, 1], F32, tag="mx", name="mx")
        nc.vector.reduce_max(mx[:], WM[ch][:], axis=mybir.AxisListType.X)
        nmx = wrk.tile([128, 1], F32, tag="nmx", name="nmx")
        nc.vector.tensor_scalar_mul(nmx[:], mx[:], -1.0)
        ex = wrk.tile([128, 9], F32, tag="ex", name="ex")
        sm = wrk.tile([128, 1], F32, tag="sm", name="sm")
        nc.scalar.activation(ex[:], WM[ch][:], mybir.ActivationFunctionType.Exp,
                             bias=nmx[:], accum_out=sm[:])
        rs = wrk.tile([128, 1], F32, tag="rs", name="rs")
        nc.vector.reciprocal(rs[:], sm[:])
        hw = csttile([128, 2], tag=f"hw{ch}")
        nc.vector.tensor_scalar_mul(hw[:], ex[:, 0:2], rs[:])
        HWH.append(hw[:, 0:1])
        base = csttile([128, 1], tag=f"base{ch}")
        nc.vector.tensor_tensor(out=base[:], in0=hw[:, 1:2], in1=FLG[:],
                                op=mybir.AluOpType.mult)
        BASE.append(base)
        b2h = csttile([128, 1], tag=f"b2h{ch}")
        nc.vector.tensor_tensor(out=b2h[:], in0=BV2[ch][:], in1=hw[:, 0:1],
                                op=mybir.AluOpType.mult)
        BV2HW.append(b2h)

    # ki_T = simil_add applied to add_keys (c2 x Lx), fp16 for fast Tadd
    KiT = [csttile([128, LX], F16, tag=f"kit{m}") for m in range(2)]
    for m in range(2):
        pps = ps_c.tile([128, LX], F32, tag="pc", name="pc")
        for dch in range(2):
            nc.tensor.matmul(pps[:], lhsT=SimT[4][dch][:, 128 * m:128 * m + 128],
                             rhs=AddkT[dch][:], start=(dch == 0), stop=(dch == 1))
        nc.vector.tensor_copy(KiT[m][:], pps[:])

    # v2 = add_keys @ W_val[2h+1].T   (Lx x C)
    V2 = [csttile([128, C], F32R, tag=f"v2{m}") for m in range(2)]
    for m in range(2):
        pps = ps_c.tile([128, C], F32, tag="pc", name="pc")
        for dch in range(2):
            nc.tensor.matmul(pps[:], lhsT=AddkT[dch][:, 128 * m:128 * m + 128],
                             rhs=Wv2T[dch][:], start=(dch == 0), stop=(dch == 1))
        nc.vector.tensor_copy(V2[m][:], pps[:])

    # add_keys scores, exp(x-16), and early unnormalized transposes
    WADD = [csttile([128, LX], tag=f"wadd{t}") for t in range(NT)]
    ZADD = [csttile([128, 1], tag=f"zadd{t}") for t in range(NT)]
    ZL = [csttile([128, 1], tag=f"zl{t}") for t in range(NT)]
    V = [csttile([128, C], F16, tag=f"v{t}") for t in range(NT)]
    WAT = [cst.tile([128, LQ], F32R, tag=f"wat{ch}", name=f"wat{ch}")
           for ch in range(2)]
    for t in range(NT):
        qsl = slice(128 * t, 128 * t + 128)
        pta = ps_c.tile([128, LX], F32, tag="pc", name="pc")
        for ch in range(2):
            nc.tensor.matmul(pta[:], lhsT=QT16[ch][:, qsl], rhs=KiT[ch][:],
                             start=(ch == 0), stop=(ch == 1))
        nc.scalar.activation(WADD[t][:], pta[:], mybir.ActivationFunctionType.Exp,
                             bias=NEG16[:], accum_out=ZADD[t][:])
        for ch in range(2):
            fsl = slice(128 * ch, 128 * ch + 128)
            tp3 = ps_tp.tile([128, 128], F32, tag="ptp", name="ptp")
            nc.tensor.transpose(out=tp3[:], in_=WADD[t][:, fsl], identity=IDENT[:])
            nc.vector.tensor_copy(WAT[ch][:, qsl], tp3[:])
        if debug and t == 0:
            nc.sync.dma_start(d_dbg_wadd[:], WADD[t][:])

    # ------- phase 3: per-level score + V accumulation ----------------------
    SALL = cst.tile([4, LQ], F32, tag="sall", name="sall")
    for lvl in range(L):
        G = G4[lvl]
        STB = [[stsb.tile([128, 512], F32R, tag=f"stb{b8}_{dch}",
                          name=f"stb{b8}_{dch}") for dch in range(2)]
               for b8 in range(2)]
        for ql in range(16):
            b, pp = ql % 4, ql // 4
            sps = ps_st.tile([128, 128], F32, tag="pst", name="pst")
            for dch in range(2):
                dsl = slice(128 * dch, 128 * dch + 128)
                for c2 in range(2):
                    nc.tensor.matmul(
                        sps[:, 64 * dch:64 * dch + 64],
                        lhsT=G[2 * b + c2][:, 256 * pp + 128 * dch:
                                 256 * pp + 128 * dch + 128],
                        rhs=QT16[c2][:].rearrange("p (a b) -> p a b", b=16)[:, :, ql],
                        start=(c2 == 0), stop=(c2 == 1))
            qb = ql % 8
            for dch in range(2):
                nc.any.tensor_copy(
                    STB[ql // 8][dch][:, 64 * qb:64 * qb + 64],
                    sps[:, 64 * dch:64 * dch + 64])
        for b8 in range(2):
            scp = ps_o.tile([4, 512], F32, tag="po", name="po")
            for ich in range(2):
                cps = ps_c.tile([128, 512], F32, tag="pc", name="pc")
                isl = slice(128 * ich, 128 * ich + 128)
                for dch in range(2):
                    nc.tensor.matmul(cps[:], lhsT=SimT[lvl][dch][:, isl],
                                     rhs=STB[b8][dch][:],
                                     start=(dch == 0), stop=(dch == 1))
                mskb = wrk.tile([128, 512], F32R, tag="mskb", name="mskb")
                nc.vector.tensor_tensor(out=mskb[:], in0=cps[:], in1=DMASK[:],
                                        op=mybir.AluOpType.mult)
                nc.tensor.matmul(scp[:], lhsT=OH48[:, 4 * ich:4 * ich + 4],
                                 rhs=mskb[:], start=(ich == 0), stop=(ich == 1))
            sview = SALL[:].rearrange("p (t s) -> p s t", s=16)
            nc.vector.tensor_copy(sview[:, 8 * b8:8 * b8 + 8, :], scp[:])

        if debug and lvl == 0:
            nc.sync.dma_start(d_dbg_sall[:], SALL[:])
        for t in range(NT):
            tps = ps_tp.tile([128, 128], F32, tag="ptp", name="ptp")
            nc.tensor.transpose(out=tps[:, :4], in_=SALL[:, 128 * t:128 * t + 128],
                                identity=IDENT[:4, :4])
            wq = wrk.tile([128, 4], F32, tag="wq", name="wq")
            zp = wrk.tile([128, 1], F32, tag="zp", name="zp")
            nc.scalar.activation(wq[:], tps[:, :4], mybir.ActivationFunctionType.Exp,
                                 bias=NEG16[:], accum_out=zp[:])
            if debug and lvl == 0:
                nc.sync.dma_start(d_dbg_wq[128 * t:128 * t + 128, :], wq[:])
            if lvl == 0:
                nc.vector.tensor_copy(ZL[t][:], zp[:])
            else:
                nc.vector.tensor_tensor(out=ZL[t][:], in0=ZL[t][:], in1=zp[:],
                                        op=mybir.AluOpType.add)
            vps = ps_v.tile([128, C], F32, tag="pv", name="pv")
            for p in range(P):
                dg = wrk.tile([128, 128], F16, tag="dg", name="dg")
                nc.vector.tensor_scalar_mul(dg[:], IDENT16[:], wq[:, p:p + 1])
                nc.tensor.matmul(vps[:], lhsT=dg[:], rhs=G[t][:, 256 * p:256 * p + 256],
                                 start=(p == 0), stop=(p == 3))
            if lvl == 0:
                nc.vector.tensor_copy(V[t][:], vps[:])
            else:
                nc.vector.tensor_tensor(out=V[t][:], in0=V[t][:], in1=vps[:],
                                        op=mybir.AluOpType.add)

    # ------- phase 4 (tail): 1/Z row, transposes, output matmuls ------------
    S1T = cst.tile([1, LQ], F32R, tag="s1t", name="s1t")
    ZR = cst.tile([1, LQ], F32, tag="zr", name="zr")
    VT = [cst.tile([128, LQ], F16, tag=f"vt{ch}", name=f"vt{ch}")
          for ch in range(2)]
    ONE1 = cst.tile([1, 128], F32, tag="one1", name="one1")
    nc.vector.memset(ONE1[:], 1.0)
    for t in range(NT):
        qsl = slice(128 * t, 128 * t + 128)
        zt = wrk.tile([128, 1], F32, tag="zt", name="zt")
        nc.vector.tensor_tensor(out=zt[:], in0=ZL[t][:], in1=ZADD[t][:],
                                op=mybir.AluOpType.add)
        if debug:
            nc.sync.dma_start(d_dbg_z[qsl, :], zt[:])
            nc.sync.dma_start(d_dbg_v[qsl, :], V[t][:])
        rz = wrk.tile([128, 1], F32, tag="rz", name="rz")
        nc.vector.reciprocal(rz[:], zt[:])
        tps = ps_tp.tile([128, 128], F32, tag="ptp", name="ptp")
        nc.tensor.transpose(out=tps[:1, :], in_=ZL[t][:], identity=IDENT[:])
        nc.scalar.copy(S1T[:, qsl], tps[:1, :])
        tpz = ps_tp.tile([128, 128], F32, tag="ptp", name="ptp")
        nc.tensor.transpose(out=tpz[:1, :], in_=rz[:], identity=IDENT[:])
        nc.scalar.copy(ZR[:, qsl], tpz[:1, :])
        for ch in range(2):
            fsl = slice(128 * ch, 128 * ch + 128)
            tp2 = ps_tp.tile([128, 128], F16, tag="ptp16", name="ptp16")
            nc.tensor.transpose(out=tp2[:], in_=V[t][:, fsl], identity=IDENT16[:])
            nc.vector.tensor_copy(VT[ch][:, qsl], tp2[:])

    RES = [cst.tile([128, LQ], F32, tag=f"res{m}", name=f"res{m}") for m in range(2)]
    for m in range(2):
        msl = slice(128 * m, 128 * m + 128)
        for n in range(2):
            nsl = slice(512 * n, 512 * n + 512)
            rzb = ps_v.tile([128, 512], F32, tag="pv", name="pv")
            nc.tensor.matmul(rzb[:], lhsT=ONE1[:], rhs=ZR[:, nsl],
                             start=True, stop=True)
            ops = ps_o.tile([128, 512], F32, tag="po", name="po")
            nc.tensor.matmul(ops[:], lhsT=WvT16[0][:, msl], rhs=VT[0][:, nsl],
                             start=True, stop=False)
            nc.tensor.matmul(ops[:], lhsT=WvT16[1][:, msl], rhs=VT[1][:, nsl],
                             start=False, stop=False)
            nc.tensor.matmul(ops[:], lhsT=BVD[:, msl], rhs=S1T[:, nsl],
                             start=False, stop=False)
            nc.tensor.matmul(ops[:], lhsT=V2[0][:, msl], rhs=WAT[0][:, nsl],
                             start=False, stop=False)
            nc.tensor.matmul(ops[:], lhsT=V2[1][:, msl], rhs=WAT[1][:, nsl],
                             start=False, stop=True)
            sc1 = wrk.tile([128, 512], F32, tag="sc1", name="sc1")
            nc.scalar.activation(sc1[:], ops[:],
                                 mybir.ActivationFunctionType.Copy, scale=HWH[m])
            nc.vector.tensor_tensor(out=sc1[:], in0=sc1[:], in1=rzb[:],
                                    op=mybir.AluOpType.mult)
            bt = wrk.tile([128, 512], F32, tag="bt", name="bt")
            nc.scalar.activation(bt[:], QTF[m][:, nsl],
                                 mybir.ActivationFunctionType.Copy, scale=BASE[m][:])
            nc.vector.tensor_tensor(out=sc1[:], in0=sc1[:], in1=bt[:],
                                    op=mybir.AluOpType.add)
            nc.vector.tensor_scalar_add(RES[m][:, nsl], sc1[:], BV2HW[m][:])
        nc.sync.dma_start(d_out[msl, :], RES[m][:])


def _host_prepare(inputs):
    """Build per-core input maps from the full problem inputs."""
    q = np.asarray(inputs["query"], np.float32)[0]            # (1024, 256)
    rp = np.asarray(inputs["reference_points"], np.float32)[0]
    flat = np.ascontiguousarray(np.asarray(inputs["input_flatten"], np.float32)[0])
    iss = np.asarray(inputs["input_spatial_shapes"], np.int32)
    addk = np.asarray(inputs["add_keys"], np.float32)[0]
    lvst = np.asarray(inputs["input_level_start_index"], np.int32)
    W_off = np.asarray(inputs["W_off"], np.float32)
    b_off = np.asarray(inputs["b_off"], np.float32)
    W_attn = np.asarray(inputs["W_attn"], np.float32)
    W_val = np.asarray(inputs["W_val"], np.float32)
    b_val = np.asarray(inputs["b_val"], np.float32)
    W_mix = np.asarray(inputs["W_mix"], np.float32)

    iss_f = iss.astype(np.float32)
    wh = iss_f[:, ::-1]                                       # (W_l, H_l)
    inv_wh32 = np.repeat((1.0 / wh)[:, None, :], P, 1).reshape(32)
    iss32 = np.repeat(iss_f[:, None, :], P, 1).reshape(32)
    hl16 = np.repeat(iss[:, 0][:, None], P, 1).reshape(16)
    lv16 = np.repeat(lvst[:, None], P, 1).reshape(16)
    rp_rep = np.repeat(rp[:, :, None, :], P, 2).reshape(LQ, 32)

    wh32 = np.repeat(wh[:, None, :], P, 1).reshape(32)
    ones128 = np.ones((128, 1), np.float32)
    common = {
        "qT": np.ascontiguousarray(q.T),
        "qT16": np.ascontiguousarray(q.T).astype(np.float16),
        "flatten16": flat.astype(np.float16),
                "invwh": np.tile(inv_wh32, (128, 8)).astype(np.float32),
        "issf": np.tile(iss32, (128, 8)).astype(np.float32),
        "wh32": np.tile(wh32, (128, 8)).astype(np.float32),
        "hl": np.tile(hl16, (128, 8)).astype(np.int32),
        "lvst": np.tile(lv16, (128, 8)).astype(np.int32),
        "addkT": np.ascontiguousarray(addk.T),
        "rp_q": np.ascontiguousarray(rp_rep.reshape(8, 128, 32).transpose(1, 0, 2).reshape(128, 256)).astype(np.float32),
    }
    # diag extraction mask: rows r=(ql%2)*64+t, cols p*64+t' -> 1 iff t'==r%64
    dm = np.zeros((128, 512), np.float32)
    for rr in range(128):
        dm[rr, rr % 64::64] = 1.0
    common["dmask"] = dm
    oh = np.zeros((128, 8), np.float32)
    for rr in range(128):
        oh[rr, rr // 64] = 1.0          # ich 0: i//64 = p
        oh[rr, 4 + 2 + rr // 64] = 1.0  # ich 1: p = 2 + i'//64
    common["oh48"] = oh

    in_maps = []
    for h in range(H):
        boff = b_off[32 * h:32 * h + 32]
        order = [h, 8] + [k for k in range(9) if k not in (h, 8)]
        m = dict(common)
        m["boff"] = np.tile(boff, (128, 8)).astype(np.float32)
        m["woffT"] = np.ascontiguousarray(W_off[32 * h:32 * h + 32].T)
        m["wattnT"] = np.ascontiguousarray(
            np.transpose(W_attn[4 * h:4 * h + 5], (0, 2, 1)))
        m["wvT16"] = np.ascontiguousarray(W_val[2 * h].T).astype(np.float16)
        m["wv2T"] = np.ascontiguousarray(W_val[2 * h + 1].T)
        m["bvd"] = (b_val[2 * h] - b_val[2 * h + 1]).reshape(1, C).astype(np.float32)
        m["bv2"] = b_val[2 * h + 1].reshape(C, 1).astype(np.float32)
        m["wmix_r"] = np.ascontiguousarray(W_mix[:, order])
        m["flag"] = ones128 * (1.0 if h == 0 else 0.0)
        in_maps.append(m)
    return in_maps


_CACHE = {}


def _get_nc():
    if "nc" not in _CACHE:
        nc = bacc.Bacc("TRN2", target_bir_lowering=False, debug=False)
        with tile.TileContext(nc) as tc:
            with ExitStack() as ctx:
                build_kernel(nc, tc, ctx)
        nc.compile()
        _CACHE["nc"] = nc
    return _CACHE["nc"]


def kernel(**inputs):
    nc = _get_nc()
    in_maps = _host_prepare(inputs)
    res = run_bass_kernel_spmd(nc, in_maps, core_ids=list(range(8)))
    total = np.zeros((C, LQ), np.float32)
    for h in range(H):
        total = total + res.results[h]["outT"]
    return np.ascontiguousarray(total.T)[None].astype(np.float32)


def debug_run(inputs, cores=(0,)):
    nc = bacc.Bacc("TRN2", target_bir_lowering=False, debug=False)
    with tile.TileContext(nc) as tc:
        with ExitStack() as ctx:
            build_kernel(nc, tc, ctx, debug=True)
    nc.compile()
    in_maps = _host_prepare(inputs)
    sel = [in_maps[c] for c in cores]
    res = run_bass_kernel_spmd(nc, sel, core_ids=list(range(len(sel))))
    return res.results


if __name__ == "__main__":
    import reference as R
    import jax.numpy as jnp

    inputs = {k: np.asarray(v) for k, v in R.setup_inputs().items()}
    out = kernel(**inputs)
    exp = np.asarray(R.reference(**{k: jnp.asarray(v) for k, v in inputs.items()}))
    err = np.abs(out - exp)
    scale = np.abs(exp).max()
    print("max abs err:", err.max(), "scale:", scale, "rel:", err.max() / scale)

